# revision 15
# baseline (speedup 1.0000x reference)
"""Trainium2 Bass kernel for the FAE (forecasting autoencoder + Koopman) module.

Strategy (8 NeuronCores, SPMD, one launch):
  - Tiny encoder conv stack: replicated on every core (negligible work).
  - efc (z = h @ efc_w.T + b, [800]->[32768]): efc_w row-sharded, 4096 rows
    per core; per-core z-shard AllGathered (16 KB/rank).
  - Koopman scalar recurrence (Af, powers): replicated on every core from the
    gathered z (vector/scalar engines + one PE broadcast matmul).
  - dfc (d = out2 @ dfc_w.T + b, [2,32767]->[2,800]): sharded on the OUTPUT
    dim m (100 outputs per core) so the SPMD program is identical across
    cores (only weight data differs); per-core [2,100] slices AllGathered.
  - Tiny decoder convT stack: replicated.
All heavy memory traffic (efc_w + dfc_w) is streamed by DMA in large
contiguous chunks as host-cast bf16 (f32 accumulation in PSUM keeps the
dot-product error ~2^-9 relative, far under tolerance, while halving both
DMA bytes and TensorEngine passes - fp32 matmuls cost 2 HW passes each).
Weights are host-prepacked into exact SBUF layouts (transposed + padded +
partition-chunked) so the device never transposes them; all ~40 small
constants ship in two packed tensors (2 DMAs) to avoid serializing the
HWDGE queues at startup.
"""

import os
import sys

import numpy as np

if "/opt/trn_rl_repo" not in sys.path:
    sys.path.insert(0, "/opt/trn_rl_repo")

NC = 8
W = 32768
F = W - 1
NSH = W // NC  # 4096 z columns per core
MSH = 800 // NC  # 100 dfc output columns per core
KP = 7  # ceil(800/128) contraction chunks for efc
EPS = 1e-5
LAM = 0.01

DEBUG = bool(int(os.environ.get("BASS_FAE_DEBUG", "0")))

_CACHE = {}

# ---- packed-constant layouts (host and device share these) ----
def _mkpack(entries):
    off = 0
    table = {}
    for name, P, cols in entries:
        table[name] = (off, cols, P)
        off += cols
    return table, off


PKF_OFF, PKF_COLS = _mkpack([
    ("e1b", 128, 2), ("bn2g", 128, 2), ("bn2b", 128, 2),
    ("e2b", 128, 1), ("bn3g", 128, 1), ("bn3b", 128, 1),
    ("e3b", 64, 1), ("bn4g", 64, 1), ("bn4b", 64, 1), ("e4b", 32, 1),
    ("d4b", 64, 1), ("dbn4g", 32, 1), ("dbn4b", 32, 1),
    ("dbn3g", 64, 1), ("dbn3b", 64, 1), ("d3b", 128, 1),
    ("dbn2g", 128, 1), ("dbn2b", 128, 1), ("d2b", 128, 2),
    ("dbn1g", 128, 2), ("dbn1b", 128, 2), ("d1b", 1, 1),
    ("ones128", 128, 1), ("onesrow", 1, 128), ("par", 128, 1),
    ("epsc", 128, 1), ("ident", 128, 128),
    ("iof0", 128, 128), ("iof1", 128, 128),
    ("e1R", 1, 768), ("e2R", 128, 768), ("e3R", 128, 192), ("e4R", 64, 96),
])
PKB_OFF, PKB_COLS = _mkpack([
    ("ones11", 1, 1), ("ones2", 1, 2), ("efcb", 1, NSH), ("dfcb", 1, MSH),
    ("d4R", 32, 320), ("d3R", 64, 640), ("d2R", 128, 1280), ("d1R", 128, 6),
    ("identb", 128, 128),
])



def _bn_scale_shift(nc, wk, x_aps, g_ap, b_ap, P, L, nblk, tag, eps_ap=None):
    """Batch-norm (training mode, biased var) per-channel scale/shift.

    x_aps: list of nblk APs, each [P, L] (channel = partition + P*blk).
    Returns (scale_ap, shift_ap), each [P, nblk]:
      out = x * scale + shift  ==  g * (x - mu) * rsqrt(var + eps) + b
    """
    import concourse.mybir as mybir

    fp = mybir.dt.float32
    sums = wk.tile([P, nblk], fp, name=f"{tag}_sums", tag=f"{tag}_sums")
    ss = wk.tile([P, nblk], fp, name=f"{tag}_ss", tag=f"{tag}_ss")
    sq = wk.tile([P, L], fp, name=f"{tag}_sq", tag=f"{tag}_sq")
    for blk in range(nblk):
        nc.vector.tensor_reduce(
            out=sums[:, blk : blk + 1],
            in_=x_aps[blk],
            axis=mybir.AxisListType.X,
            op=mybir.AluOpType.add,
        )
        nc.scalar.activation(
            out=sq[:, :],
            in_=x_aps[blk],
            func=mybir.ActivationFunctionType.Square,
            accum_out=ss[:, blk : blk + 1],
        )
    # mu = sums/L ; ex2 = ss/L ; var = ex2 - mu^2
    nc.vector.tensor_scalar_mul(sums[:, :], sums[:, :], 1.0 / L)  # sums <- mu
    nc.vector.tensor_scalar_mul(ss[:, :], ss[:, :], 1.0 / L)  # ss <- E[x^2]
    musq = wk.tile([P, nblk], fp, name=f"{tag}_musq", tag=f"{tag}_musq")
    nc.vector.tensor_mul(musq[:, :], sums[:, :], sums[:, :])
    nc.vector.tensor_sub(ss[:, :], ss[:, :], musq[:, :])  # ss <- var
    # sd = sqrt(var + eps); inv = 1/sd
    nc.scalar.activation(
        out=ss[:, :], in_=ss[:, :], func=mybir.ActivationFunctionType.Sqrt,
        bias=eps_ap,
    )
    nc.vector.reciprocal(musq[:, :], ss[:, :])  # musq <- rsqrt(var+eps)
    # scale = inv * g ; shift = b - mu*scale
    nc.vector.tensor_mul(musq[:, :], musq[:, :], g_ap)  # musq <- scale
    nc.vector.tensor_mul(sums[:, :], sums[:, :], musq[:, :])  # sums <- mu*scale
    shift = wk.tile([P, nblk], fp, name=f"{tag}_shift", tag=f"{tag}_shift")
    nc.vector.tensor_sub(shift[:, :], b_ap, sums[:, :])
    return musq, shift


def _build(debug=False):
    import concourse.bass as bass  # noqa: F401
    import concourse.mybir as mybir
    import concourse.tile as tile
    from concourse import bacc

    fp = mybir.dt.float32
    bf = mybir.dt.bfloat16
    i32 = mybir.dt.int32

    nc = bacc.Bacc(
        "TRN2",
        target_bir_lowering=False,
        debug=False,
        enable_asserts=False,
        num_devices=NC,
    )

    def din(name, shape, dt=None):
        return nc.dram_tensor(
            name, list(shape), dt or fp, kind="ExternalInput"
        ).ap()

    # ---- inputs ----
    # All small constants are packed into two tensors (one f32, one bf16)
    # so they arrive in 2 DMAs instead of ~40 serialized ones.
    x_d = din("x", (1, 33))
    pkf_d = din("pkf", (128, PKF_COLS))
    pkb_d = din("pkb", (128, PKB_COLS), bf)
    w1r_d = din("w1r", (128, KP, NSH), bf)  # per-core efc shard (k-chunk, n)
    w2r_d = din("w2r", (128, 256, MSH), bf)  # per-core dfc m-shard (c-chunk, m)
    out_d = nc.dram_tensor("out", [1, 64], fp, kind="ExternalOutput").ap()
    if debug:
        zdbg_d = nc.dram_tensor("zdbg", [8, 4096], fp, kind="ExternalOutput").ap()
        ddbg_d = nc.dram_tensor("ddbg", [2, 800], fp, kind="ExternalOutput").ap()

    rg = [list(range(NC))]

    with tile.TileContext(nc) as tc:
        with (
            tc.tile_pool(name="const", bufs=1) as cst,
            tc.tile_pool(name="w1pool", bufs=7) as w1p,
            tc.tile_pool(name="w2pool", bufs=8) as w2p,
            tc.tile_pool(name="work", bufs=1) as wk_,
            tc.tile_pool(name="drampool", bufs=1, space="DRAM") as dr,
        ):
            class WK:
                """wrapper that always passes a distinct tag"""

                def __init__(self, pool):
                    self.pool = pool

                def tile(self, shape, dtype, name=None, tag=None):
                    assert name or tag
                    tag = tag or name
                    name = name or tag
                    return self.pool.tile(shape, dtype, name=name, tag=tag)

            wk = WK(wk_)

            # ---- load packed constants (2 DMAs) ----
            x_sb = cst.tile([1, 33], fp, name="x_sb", tag="x_sb")
            nc.sync.dma_start(x_sb[...], x_d)
            pkf = cst.tile([128, PKF_COLS], fp, name="pkf_sb", tag="pkf_sb")
            nc.sync.dma_start(pkf[...], pkf_d)
            pkb = cst.tile([128, PKB_COLS], bf, name="pkb_sb", tag="pkb_sb")
            nc.sync.dma_start(pkb[...], pkb_d)

            def vf(name, reshape=None):
                off, cols, P = PKF_OFF[name]
                ap = pkf[0:P, off : off + cols]
                if reshape:
                    ap = ap.rearrange(*reshape[0], **reshape[1])
                return ap

            def vb(name, reshape=None):
                off, cols, P = PKB_OFF[name]
                ap = pkb[0:P, off : off + cols]
                if reshape:
                    ap = ap.rearrange(*reshape[0], **reshape[1])
                return ap

            e1R = vf("e1R", (("p (dk o) -> p dk o",), dict(dk=3)))
            e1b = vf("e1b")
            bn2g = vf("bn2g")
            bn2b = vf("bn2b")
            e2R = vf("e2R", (("p (dk ib o) -> p dk ib o",), dict(dk=3, ib=2)))
            e2b = vf("e2b")
            bn3g = vf("bn3g")
            bn3b = vf("bn3b")
            e3R = vf("e3R", (("p (dk o) -> p dk o",), dict(dk=3)))
            e3b = vf("e3b")
            bn4g = vf("bn4g")
            bn4b = vf("bn4b")
            e4R = vf("e4R", (("p (dk o) -> p dk o",), dict(dk=3)))
            e4b = vf("e4b")
            d4b = vf("d4b")
            dbn4g = vf("dbn4g")
            dbn4b = vf("dbn4b")
            dbn3g = vf("dbn3g")
            dbn3b = vf("dbn3b")
            d3b = vf("d3b")
            dbn2g = vf("dbn2g")
            dbn2b = vf("dbn2b")
            d2b = vf("d2b")
            dbn1g = vf("dbn1g")
            dbn1b = vf("dbn1b")
            d1b = vf("d1b")
            ones128 = vf("ones128")
            onesrow = vf("onesrow")
            par = vf("par")
            epsc = vf("epsc")
            ident = vf("ident")
            iof0 = vf("iof0")
            iof1 = vf("iof1")
            ones11 = vb("ones11")
            ones2 = vb("ones2")
            efcb = vb("efcb")
            dfcb = vb("dfcb")
            d4R = vb("d4R", (("p (dk o) -> p dk o",), dict(dk=5)))
            d3R = vb("d3R", (("p (dk o) -> p dk o",), dict(dk=5)))
            d2R = vb("d2R", (("p (dk ob o) -> p dk ob o",), dict(dk=5, ob=2)))
            d1R = vb("d1R", (("p (dk ib o) -> p dk ib o",), dict(dk=3, ib=2)))
            identb = vb("identb")

            AF = mybir.ActivationFunctionType
            AL = mybir.AluOpType

            def scope(name):
                sid, _ = nc.enter_named_scope(name, False)
                return (name, sid)

            def unscope(s):
                nc.leave_named_scope(s[0], s[1], False)

            # =========== ENCODER (replicated) ===========
            _s = scope("enc")
            with tc.tile_pool(name="psE", bufs=2, space="PSUM") as psE:
                # L1: [1,33] -> [256ch, 31] as [128, 2, 31]
                h1 = wk.tile([128, 2, 31], fp, "h1")
                for blk in range(2):
                    ps = psE.tile([128, 31], fp, name="cps", tag="cps")
                    for dk in range(3):
                        nc.tensor.matmul(
                            ps[:, :],
                            e1R[0:1, dk, blk * 128 : (blk + 1) * 128],
                            x_sb[0:1, dk : dk + 31],
                            start=(dk == 0),
                            stop=(dk == 2),
                        )
                    nc.vector.tensor_scalar(
                        h1[:, blk, :], ps[:, :], e1b[:, blk : blk + 1], None, AL.add
                    )
                # BN2 + relu
                sc, sh = _bn_scale_shift(
                    nc, wk, [h1[:, 0, :], h1[:, 1, :]], bn2g[:, :], bn2b[:, :],
                    128, 31, 2, "bn2", eps_ap=epsc[0:128, 0:1],
                )
                a2 = wk.tile([128, 2, 31], fp, "a2")
                for blk in range(2):
                    nc.scalar.activation(
                        a2[:, blk, :], h1[:, blk, :], AF.Relu,
                        bias=sh[:, blk : blk + 1], scale=sc[:, blk : blk + 1],
                    )
                # L2: -> [128, 29]
                h2 = wk.tile([128, 29], fp, "h2")
                ps = psE.tile([128, 29], fp, name="cps", tag="cps")
                n_mm = 0
                for dk in range(3):
                    for ib in range(2):
                        nc.tensor.matmul(
                            ps[:, :],
                            e2R[:, dk, ib, :],
                            a2[:, ib, dk : dk + 29],
                            start=(n_mm == 0),
                            stop=(n_mm == 5),
                        )
                        n_mm += 1
                nc.vector.tensor_scalar(h2[:, :], ps[:, :], e2b[:, :], None, AL.add)
                # BN3 + relu
                sc, sh = _bn_scale_shift(
                    nc, wk, [h2[:, :]], bn3g[:, :], bn3b[:, :], 128, 29, 1, "bn3", eps_ap=epsc[0:128, 0:1]
                )
                a3 = wk.tile([128, 29], fp, "a3")
                nc.scalar.activation(
                    a3[:, :], h2[:, :], AF.Relu, bias=sh[:, :], scale=sc[:, :]
                )
                # L3: -> [64, 27]
                h3 = wk.tile([64, 27], fp, "h3")
                ps = psE.tile([64, 27], fp, name="cps", tag="cps")
                for dk in range(3):
                    nc.tensor.matmul(
                        ps[:, :],
                        e3R[:, dk, :],
                        a3[:, dk : dk + 27],
                        start=(dk == 0),
                        stop=(dk == 2),
                    )
                nc.vector.tensor_scalar(h3[:, :], ps[:, :], e3b[:, :], None, AL.add)
                # relu THEN bn4
                r4 = wk.tile([64, 27], fp, "r4")
                nc.vector.tensor_scalar_max(r4[:, :], h3[:, :], 0.0)
                sc, sh = _bn_scale_shift(
                    nc, wk, [r4[:, :]], bn4g[:, :], bn4b[:, :], 64, 27, 1, "bn4", eps_ap=epsc[0:64, 0:1]
                )
                a4 = wk.tile([64, 27], fp, "a4")
                nc.scalar.activation(
                    a4[:, :], r4[:, :], AF.Identity, bias=sh[:, :], scale=sc[:, :]
                )
                # L4: -> [32, 25]
                h4 = wk.tile([32, 25], fp, "h4")
                ps = psE.tile([32, 25], fp, name="cps", tag="cps")
                for dk in range(3):
                    nc.tensor.matmul(
                        ps[:, :],
                        e4R[:, dk, :],
                        a4[:, dk : dk + 25],
                        start=(dk == 0),
                        stop=(dk == 2),
                    )
                nc.vector.tensor_scalar(h4[:, :], ps[:, :], e4b[:, :], None, AL.add)
                # flatten to [1, 800] via DRAM bounce (row-major == channel-major)
                hflat = dr.tile([32, 25], fp, name="hflat", tag="hflat")
                nc.sync.dma_start(hflat[:, :], h4[:, :])
                hrow = wk.tile([1, 800], fp, "hrow")
                nc.sync.dma_start(
                    hrow[0:1, :], hflat.rearrange("a b -> (a b)")[None, :]
                )
                # transpose to hT [128, 7] (k on partitions, chunked)
                hT = wk.tile([128, KP], fp, "hT")
                nc.vector.memset(hT[:, :], 0.0)
                for t in range(KP):
                    w = 128 if t < 6 else 32
                    tp = psE.tile([128, 1], fp, name="trps", tag="trps")
                    nc.tensor.transpose(
                        tp[0:w, 0:1], hrow[0:1, t * 128 : t * 128 + w], ident[0:1, 0:1]
                    )
                    nc.vector.tensor_copy(hT[0:w, t : t + 1], tp[0:w, 0:1])
                hTb = wk.tile([128, KP], bf, "hTb")
                nc.vector.tensor_copy(hTb[:, :], hT[:, :])
            unscope(_s)

            # =========== EFC z-shard (tensor-parallel) ===========
            _s = scope("efc")
            z_sb = wk.tile([1, NSH], bf, "z_sb")
            with tc.tile_pool(name="psZ", bufs=8, space="PSUM") as psZ:
                zps = [
                    psZ.tile([1, 512], fp, name=f"zps{b}", tag="zps") for b in range(8)
                ]
                for b in range(8):
                    # seed with bias via K=1 matmul: out = 1 * bias_row
                    nc.tensor.matmul(
                        zps[b][:, :],
                        ones11[0:1, 0:1],
                        efcb[0:1, b * 512 : (b + 1) * 512],
                        start=True,
                        stop=False,
                    )
                for t in range(KP):
                    kw = 128 if t < 6 else 32
                    w1t = w1p.tile([128, NSH], bf, name="w1t", tag="w1t")
                    # HWDGE (Act ring): weight streaming must not sit on the
                    # gpsimd SWDGE queue behind the collectives (strict FIFO).
                    nc.scalar.dma_start(w1t[:, :], w1r_d[:, t, :])
                    for b in range(8):
                        nc.tensor.matmul(
                            zps[b][:, :],
                            hTb[0:kw, t : t + 1],
                            w1t[0:kw, b * 512 : (b + 1) * 512],
                            start=False,
                            stop=(t == KP - 1),
                        )
                for b in range(8):
                    if b % 2 == 0:
                        nc.vector.tensor_copy(
                            z_sb[0:1, b * 512 : (b + 1) * 512], zps[b][:, :]
                        )
                    else:
                        nc.scalar.copy(
                            z_sb[0:1, b * 512 : (b + 1) * 512], zps[b][:, :]
                        )

            unscope(_s)

            # ---- AllGather z ----
            _s = scope("zgather")
            cc_in = dr.tile([1, NSH], bf, name="cc_in", tag="cc_in")
            cc_out = dr.tile(
                [NC, NSH], bf, name="cc_out", tag="cc_out", addr_space="Shared"
            )
            nc.sync.dma_start(cc_in[:, :], z_sb[0:1, :])
            nc.gpsimd.collective_compute(
                "AllGather",
                mybir.AluOpType.bypass,
                replica_groups=rg,
                ins=[cc_in[:, :].opt()],
                outs=[cc_out[:, :].opt()],
            )
            if debug:
                nc.gpsimd.dma_start(zdbg_d, cc_out[:, :])

            zflat = cc_out.rearrange("r n -> (r n)")
            unscope(_s)

            # =========== Koopman scalars + out2T (replicated) ===========
            _s = scope("koop")
            z2 = wk.tile([128, 256], bf, "z2")
            zs = wk.tile([128, 256], bf, "zs")
            nc.vector.memset(zs[:, :], 0.0)
            nc.sync.dma_start(z2[:, :], zflat.rearrange("(p f) -> p f", f=256))
            nc.sync.dma_start(
                zs[0:127, :], zflat[1:32513].rearrange("(p f) -> p f", f=256)
            )
            nc.sync.dma_start(zs[127:128, 0:255], zflat[None, 32513:32768])
            z0 = wk.tile([1, 1], bf, "z0")
            zl = wk.tile([1, 1], bf, "zl")
            nc.sync.dma_start(z0[:, :], zflat[None, 0:1])
            nc.sync.dma_start(zl[:, :], zflat[None, 32767:32768])

            scr1 = wk.tile([128, 256], fp, "scr1")
            scr2 = wk.tile([128, 256], fp, "scr2")
            xyp = wk.tile([128, 1], fp, "xyp")
            xxp = wk.tile([128, 1], fp, "xxp")
            nc.vector.tensor_mul(scr1[:, :], z2[:, :], zs[:, :])
            nc.vector.tensor_reduce(
                out=xyp[:, :], in_=scr1[:, :], axis=mybir.AxisListType.X, op=AL.add
            )
            nc.scalar.activation(
                out=scr2[:, :], in_=z2[:, :], func=AF.Square, accum_out=xxp[:, :]
            )

            out2T = wk.tile([128, 2, 256], bf, "out2T")
            with tc.tile_pool(name="psS", bufs=1, space="PSUM") as psS:
                ps_xy = psS.tile([1, 1], fp, name="ps_xy", tag="ps_xy")
                ps_xx = psS.tile([1, 1], fp, name="ps_xx", tag="ps_xx")
                nc.tensor.matmul(
                    ps_xy[:, :], xyp[:, :], ones128[:, :], start=True, stop=True
                )
                nc.tensor.matmul(
                    ps_xx[:, :], xxp[:, :], ones128[:, :], start=True, stop=True
                )
                xy = wk.tile([1, 1], fp, "xy")
                xx = wk.tile([1, 1], fp, "xx")
                nc.vector.tensor_copy(xy[:, :], ps_xy[:, :])
                nc.vector.tensor_copy(xx[:, :], ps_xx[:, :])
                # Af = XY / (XX - z_last^2 + LAM)
                zl2 = wk.tile([1, 1], fp, "zl2")
                nc.scalar.activation(zl2[:, :], zl[:, :], AF.Square)
                nc.vector.tensor_sub(xx[:, :], xx[:, :], zl2[:, :])
                nc.scalar.activation(xx[:, :], xx[:, :], AF.Copy, bias=LAM)
                rec = wk.tile([1, 1], fp, "rec")
                nc.vector.reciprocal(rec[:, :], xx[:, :])
                af = wk.tile([1, 1], fp, "af")
                nc.vector.tensor_mul(af[:, :], xy[:, :], rec[:, :])
                # L = ln|Af| ; aneg = (Af < 0)
                laf = wk.tile([1, 1], fp, "laf")
                nc.scalar.activation(laf[:, :], af[:, :], AF.Abs)
                nc.scalar.activation(laf[:, :], laf[:, :], AF.Ln)
                aneg = wk.tile([1, 1], fp, "aneg")
                nc.vector.tensor_scalar(aneg[:, :], af[:, :], 0.0, None, AL.is_lt)
                # broadcast (L, z0, aneg) to all partitions via K=1 matmul
                srow = wk.tile([1, 3], fp, "srow")
                nc.vector.tensor_copy(srow[0:1, 0:1], laf[:, :])
                nc.vector.tensor_copy(srow[0:1, 1:2], z0[:, :])
                nc.vector.tensor_copy(srow[0:1, 2:3], aneg[:, :])
                ps_bc = psS.tile([128, 3], fp, name="ps_bc", tag="ps_bc")
                nc.tensor.matmul(
                    ps_bc[:, :], onesrow[0:1, :], srow[0:1, :], start=True, stop=True
                )
                bc = wk.tile([128, 3], fp, "bc")
                nc.vector.tensor_copy(bc[:, :], ps_bc[:, :])
                # coef = X0 * (1 - 2*aneg*parity)
                coef = wk.tile([128, 1], fp, "coef")
                nc.vector.tensor_mul(coef[:, :], bc[:, 2:3], par[:, :])
                nc.vector.tensor_scalar(
                    coef[:, :], coef[:, :], -2.0, 1.0, AL.mult, AL.add
                )
                nc.vector.tensor_mul(coef[:, :], coef[:, :], bc[:, 1:2])
                # powers, partition-minor: out2T[q, 1, u] = coef[q]*exp(L*(256u+q))
                # iof0/iof1 are host-precomputed iota tables (256u+q, 256u+q+128)
                pe0 = wk.tile([128, 128], fp, "pe0")
                pe1 = wk.tile([128, 128], fp, "pe1")
                nc.scalar.activation(pe0[:, :], iof0, AF.Exp, scale=bc[:, 0:1])
                nc.scalar.activation(pe1[:, :], iof1, AF.Exp, scale=bc[:, 0:1])
                nc.vector.tensor_scalar(
                    out2T[:, 1, 0:128], pe0[:, :], coef[:, :], None, AL.mult
                )
                nc.vector.tensor_scalar(
                    out2T[:, 1, 128:256], pe1[:, :], coef[:, :], None, AL.mult
                )
                # X, partition-minor: transpose z2 in two 128x128 blocks
                psT0 = psS.tile([128, 128], bf, name="psT0", tag="psT0")
                psT1 = psS.tile([128, 128], bf, name="psT1", tag="psT1")
                nc.tensor.transpose(psT0[:, :], z2[:, 0:128], identb[:, :])
                nc.tensor.transpose(psT1[:, :], z2[:, 128:256], identb[:, :])
                nc.vector.tensor_copy(out2T[:, 0, 0:128], psT0[:, :])
                nc.vector.tensor_copy(out2T[:, 0, 128:256], psT1[:, :])

            unscope(_s)

            # =========== DFC (m-sharded) ===========
            _s = scope("dfc")
            d_sb = wk.tile([2, MSH], fp, "d_sb")
            with tc.tile_pool(name="psD1", bufs=1, space="PSUM") as psD1:
                dps = psD1.tile([2, MSH], fp, name="dps", tag="dps")
                nc.tensor.matmul(
                    dps[:, :], ones2[0:1, :], dfcb[0:1, :], start=True, stop=False
                )
                for ci in range(8):
                    w2t = w2p.tile([128, 32, MSH], bf, name="w2t", tag="w2t")
                    nc.scalar.dma_start(
                        w2t[:, :, :], w2r_d[:, ci * 32 : (ci + 1) * 32, :]
                    )
                    for tt in range(32):
                        tg = ci * 32 + tt
                        off = 128 * (tg & 1) + (tg >> 1)
                        nc.tensor.matmul(
                            dps[:, :],
                            out2T[:, :, off],
                            w2t[:, tt, :],
                            start=False,
                            stop=(tg == 255),
                        )
                nc.vector.tensor_copy(d_sb[:, :], dps[:, :])

            unscope(_s)

            # ---- AllGather d slices ----
            _s = scope("dgather")
            cc2_in = dr.tile([1, 2 * MSH], fp, name="cc2_in", tag="cc2_in")
            cc2_out = dr.tile(
                [NC, 2 * MSH], fp, name="cc2_out", tag="cc2_out", addr_space="Shared"
            )
            nc.sync.dma_start(
                cc2_in.rearrange("a (r m) -> (a r) m", r=2, m=MSH), d_sb[:, :]
            )
            nc.gpsimd.collective_compute(
                "AllGather",
                mybir.AluOpType.bypass,
                replica_groups=rg,
                ins=[cc2_in[:, :].opt()],
                outs=[cc2_out[:, :].opt()],
            )
            if debug:
                nc.sync.dma_start(
                    ddbg_d.rearrange("r (j m) -> r j m", j=8), cc2_out.rearrange("j (r m) -> r j m", r=2)
                )

            unscope(_s)

            # =========== DECODER (replicated) ===========
            _s = scope("dec")
            with tc.tile_pool(name="psD2", bufs=2, space="PSUM") as psD2:
                # d.reshape(32, 50): ch = 16r + 2j + h, l  (m = 100j + 50h + l)
                D4 = wk.tile([32, 50], fp, "D4")
                cc2v = cc2_out.rearrange("j (r h l) -> r j h l", r=2, h=2, l=50)
                for r in range(2):
                    nc.sync.dma_start(D4[16 * r : 16 * (r + 1), :], cc2v[r])
                # bn4 then relu
                sc, sh = _bn_scale_shift(
                    nc, wk, [D4[:, :]], dbn4g[:, :], dbn4b[:, :], 32, 50, 1, "dbn4", eps_ap=epsc[0:32, 0:1]
                )
                ap4 = wk.tile([32, 58], bf, "ap4")
                nc.vector.memset(ap4[:, :], 0.0)
                nc.scalar.activation(
                    ap4[:, 4:54], D4[:, :], AF.Relu, bias=sh[:, :], scale=sc[:, :]
                )
                # convT4 -> [64, 54]
                h5 = wk.tile([64, 54], fp, "h5")
                ps = psD2.tile([64, 54], fp, name="dcps", tag="dcps")
                for dk in range(5):
                    nc.tensor.matmul(
                        ps[:, :],
                        d4R[:, dk, :],
                        ap4[:, dk : dk + 54],
                        start=(dk == 0),
                        stop=(dk == 4),
                    )
                nc.vector.tensor_scalar(h5[:, :], ps[:, :], d4b[:, :], None, AL.add)
                # relu then bn3
                nc.vector.tensor_scalar_max(h5[:, :], h5[:, :], 0.0)
                sc, sh = _bn_scale_shift(
                    nc, wk, [h5[:, :]], dbn3g[:, :], dbn3b[:, :], 64, 54, 1, "dbn3", eps_ap=epsc[0:64, 0:1]
                )
                ap3 = wk.tile([64, 62], bf, "ap3")
                nc.vector.memset(ap3[:, :], 0.0)
                nc.scalar.activation(
                    ap3[:, 4:58], h5[:, :], AF.Identity, bias=sh[:, :], scale=sc[:, :]
                )
                # convT3 -> [128, 58]
                h6 = wk.tile([128, 58], fp, "h6")
                ps = psD2.tile([128, 58], fp, name="dcps", tag="dcps")
                for dk in range(5):
                    nc.tensor.matmul(
                        ps[:, :],
                        d3R[:, dk, :],
                        ap3[:, dk : dk + 58],
                        start=(dk == 0),
                        stop=(dk == 4),
                    )
                nc.vector.tensor_scalar(h6[:, :], ps[:, :], d3b[:, :], None, AL.add)
                # relu then bn2
                nc.vector.tensor_scalar_max(h6[:, :], h6[:, :], 0.0)
                sc, sh = _bn_scale_shift(
                    nc, wk, [h6[:, :]], dbn2g[:, :], dbn2b[:, :], 128, 58, 1, "dbn2", eps_ap=epsc[0:128, 0:1]
                )
                ap2 = wk.tile([128, 66], bf, "ap2")
                nc.vector.memset(ap2[:, :], 0.0)
                nc.scalar.activation(
                    ap2[:, 4:62], h6[:, :], AF.Identity, bias=sh[:, :], scale=sc[:, :]
                )
                # convT2 -> [256, 62] as [128, 2, 62]
                h7 = wk.tile([128, 2, 62], fp, "h7")
                for ob in range(2):
                    ps = psD2.tile([128, 62], fp, name="dcps", tag="dcps")
                    for dk in range(5):
                        nc.tensor.matmul(
                            ps[:, :],
                            d2R[:, dk, ob, :],
                            ap2[:, dk : dk + 62],
                            start=(dk == 0),
                            stop=(dk == 4),
                        )
                    nc.vector.tensor_scalar(
                        h7[:, ob, :], ps[:, :], d2b[:, ob : ob + 1], None, AL.add
                    )
                # bn1 then relu
                sc, sh = _bn_scale_shift(
                    nc, wk, [h7[:, 0, :], h7[:, 1, :]], dbn1g[:, :], dbn1b[:, :],
                    128, 62, 2, "dbn1", eps_ap=epsc[0:128, 0:1],
                )
                ap1 = wk.tile([128, 2, 66], bf, "ap1")
                nc.vector.memset(ap1[:, :, :], 0.0)
                for ob in range(2):
                    nc.scalar.activation(
                        ap1[:, ob, 2:64], h7[:, ob, :], AF.Relu,
                        bias=sh[:, ob : ob + 1], scale=sc[:, ob : ob + 1],
                    )
                # convT1 -> [1, 64]
                ps = psD2.tile([1, 64], fp, name="fps", tag="fps")
                n_mm = 0
                for dk in range(3):
                    for ib in range(2):
                        nc.tensor.matmul(
                            ps[:, :],
                            d1R[:, dk, ib, 0:1],
                            ap1[:, ib, dk : dk + 64],
                            start=(n_mm == 0),
                            stop=(n_mm == 5),
                        )
                        n_mm += 1
                o_sb = wk.tile([1, 64], fp, "o_sb")
                nc.vector.tensor_scalar(o_sb[:, :], ps[:, :], d1b[:, :], None, AL.add)
                nc.sync.dma_start(out_d, o_sb[0:1, :])
            unscope(_s)

    nc.compile()
    return nc


def _repack(inputs):
    """Host-side layout prep: shard + transpose weights into packed layouts."""
    import ml_dtypes

    f32 = np.float32
    bf16 = ml_dtypes.bfloat16

    def arr(k):
        return np.asarray(inputs[k], dtype=f32)

    vals_f = {}
    vals_f["e1b"] = arr("e1_b").reshape(2, 128).T
    vals_f["bn2g"] = arr("bn2_g").reshape(2, 128).T
    vals_f["bn2b"] = arr("bn2_b").reshape(2, 128).T
    vals_f["e2b"] = arr("e2_b").reshape(128, 1)
    vals_f["bn3g"] = arr("bn3_g").reshape(128, 1)
    vals_f["bn3b"] = arr("bn3_b").reshape(128, 1)
    vals_f["e3b"] = arr("e3_b").reshape(64, 1)
    vals_f["bn4g"] = arr("bn4_g").reshape(64, 1)
    vals_f["bn4b"] = arr("bn4_b").reshape(64, 1)
    vals_f["e4b"] = arr("e4_b").reshape(32, 1)
    vals_f["d4b"] = arr("d4_b").reshape(64, 1)
    vals_f["dbn4g"] = arr("dbn4_g").reshape(32, 1)
    vals_f["dbn4b"] = arr("dbn4_b").reshape(32, 1)
    vals_f["dbn3g"] = arr("dbn3_g").reshape(64, 1)
    vals_f["dbn3b"] = arr("dbn3_b").reshape(64, 1)
    vals_f["d3b"] = arr("d3_b").reshape(128, 1)
    vals_f["dbn2g"] = arr("dbn2_g").reshape(128, 1)
    vals_f["dbn2b"] = arr("dbn2_b").reshape(128, 1)
    vals_f["d2b"] = arr("d2_b").reshape(2, 128).T
    vals_f["dbn1g"] = arr("dbn1_g").reshape(2, 128).T
    vals_f["dbn1b"] = arr("dbn1_b").reshape(2, 128).T
    vals_f["d1b"] = arr("d1_b").reshape(1, 1)
    vals_f["ones128"] = np.ones((128, 1), f32)
    vals_f["onesrow"] = np.ones((1, 128), f32)
    vals_f["par"] = (np.arange(128, dtype=f32) % 2).reshape(128, 1)
    vals_f["epsc"] = np.full((128, 1), EPS, f32)
    vals_f["ident"] = np.eye(128, dtype=f32)
    io = np.arange(128, dtype=f32)[:, None] + 256.0 * np.arange(128, dtype=f32)[None, :]
    vals_f["iof0"] = io
    vals_f["iof1"] = io + 128.0
    vals_f["e1R"] = arr("e1_w")[:, 0, :].T.reshape(1, 768)  # [1, dk*o]
    vals_f["e2R"] = (
        arr("e2_w").transpose(2, 1, 0).reshape(3, 2, 128, 128)
        .transpose(2, 0, 1, 3).reshape(128, 768)
    )
    vals_f["e3R"] = arr("e3_w").transpose(1, 2, 0).reshape(128, 192)
    vals_f["e4R"] = arr("e4_w").transpose(1, 2, 0).reshape(64, 96)

    pkf = np.zeros((128, PKF_COLS), f32)
    for name, (off, cols, P) in PKF_OFF.items():
        pkf[:P, off : off + cols] = vals_f[name]

    vals_b = {}
    vals_b["ones11"] = np.ones((1, 1), f32)
    vals_b["ones2"] = np.ones((1, 2), f32)
    vals_b["d4R"] = np.flip(arr("d4_w"), -1).transpose(0, 2, 1).reshape(32, 320)
    vals_b["d3R"] = np.flip(arr("d3_w"), -1).transpose(0, 2, 1).reshape(64, 640)
    vals_b["d2R"] = (
        np.flip(arr("d2_w"), -1).transpose(0, 2, 1).reshape(128, 1280)
    )
    vals_b["identb"] = np.eye(128, dtype=f32)
    vals_b["d1R"] = (
        np.flip(arr("d1_w"), -1).transpose(0, 2, 1).reshape(2, 128, 3)
        .transpose(1, 2, 0).reshape(128, 6)
    )

    # efc: [32768, 800] -> transpose -> pad k to 896 -> [8, 128, 7, 4096]
    w1t = arr("efc_w").T  # [800, 32768]
    w1p = np.zeros((KP * 128, W), f32)
    w1p[:800] = w1t
    w1r = np.ascontiguousarray(
        w1p.reshape(KP, 128, NC, NSH).transpose(2, 1, 0, 3)
    ).astype(bf16)  # [8, 128, 7, 4096]
    efcb = arr("efc_b").reshape(NC, 1, NSH)

    # dfc: [800, 32767] -> transpose -> pad c to 32768 -> [8, 128, 256, 100]
    w2t = np.zeros((W, 800), f32)
    w2t[:F] = arr("dfc_w").T
    w2r = np.ascontiguousarray(
        w2t.reshape(256, 128, NC, MSH).transpose(2, 1, 0, 3)
    ).astype(bf16)  # [8, 128, 256, 100]
    dfcb = arr("dfc_b").reshape(NC, 1, MSH)

    x = arr("x").reshape(1, 33)
    in_maps = []
    for j in range(NC):
        vb = dict(vals_b)
        vb["efcb"] = efcb[j]
        vb["dfcb"] = dfcb[j]
        pkb = np.zeros((128, PKB_COLS), f32)
        for name, (off, cols, P) in PKB_OFF.items():
            pkb[:P, off : off + cols] = vb[name]
        in_maps.append({
            "x": x,
            "pkf": pkf,
            "pkb": pkb.astype(bf16),
            "w1r": w1r[j],
            "w2r": w2r[j],
        })
    return in_maps


def get_nc(debug=DEBUG):
    key = ("nc", debug)
    if key not in _CACHE:
        _CACHE[key] = _build(debug=debug)
    return _CACHE[key]


def run(inputs, debug=DEBUG, **kwargs):
    from concourse.bass_utils import run_bass_kernel_spmd

    nc = get_nc(debug=debug)
    in_maps = _repack(inputs)
    res = run_bass_kernel_spmd(nc, in_maps, core_ids=list(range(NC)), **kwargs)
    return res


def kernel(**inputs):
    res = run(inputs)
    return res.results[0]["out"].reshape(64).astype(np.float32)

